# revision 3
# baseline (speedup 1.0000x reference)
"""Trainium2 Bass kernel for GQA attention (B=8, S=1024, H=2048, 32 Q / 8 KV heads, D=64).

Data-parallel over batch: one batch element per NeuronCore, weights replicated,
zero collectives. All PE matmuls in bf16 (host pre-casts inputs to bf16).

Per-core pipeline:
  1. Per 128-row hidden tile t: PE-transpose (identity matmul) into hT
     [H, S] bf16, immediately followed by the V-projection tile st=t
     (stationary hT chunk, moving Wv) -> va [s, 8*65] (64 v cols + a ones
     column per KV group).
  2. One software-pipelined window over 32 head-slots. Each slot emits the
     head's scores (causal 128-key tiles, ScalarE exp -> ex bf16, Pool
     triangle-mask multiply on the diagonal tile), the previous head's PV
     (ex tiles stationary, v_aug moving: 65 rows per 128x128 tile; the
     ones column accumulates the softmax denominator per query partition),
     DVE reciprocal + per-partition-scalar normalize,
          and XBAR DMA-transposes of the normalized output into attT [hd, i]
     (all transposes share the sync queue — two XBAR transposes in flight
     on different queues corrupt each other). The K projection (+RoPE, kT
     slot duplication) and Q projection (+RoPE, qT stays in SBUF) for
     group g+1 are spread across group g's slots as PE filler, which keeps
     the ScalarE exp stream (the only engine with exp; ~170us total) fully
     hidden under PE work. The first two O-projection psum tiles
     accumulate as tail filler in slots 30-31.
  3. O projection (bf16) with streamed Wo chunks; the scores PSUM pool is
     released at slot 31 to make room for its psum tiles.

Cost-model schedule (what neuron-profile would measure): ~389 us/core,
~95% PE occupancy (PE busy ~374 us, which is the matmul-throughput floor
for this dataflow). Relative error vs the f32 reference: ~6.2e-3.
"""

import contextlib

import numpy as np
import ml_dtypes

import concourse.bass as bass
import concourse.tile as tile
from concourse import bacc, mybir
from concourse.bass_utils import run_bass_kernel_spmd

B, S, H = 8, 1024, 2048
NQ, NKV, D = 32, 8, 64
F32 = mybir.dt.float32
BF16 = mybir.dt.bfloat16
AF = mybir.ActivationFunctionType
BF = ml_dtypes.bfloat16


def _tables():
    inv = 1.0 / (10000.0 ** (np.arange(0, D, 2, dtype=np.float64) / D))  # [32]
    fr = np.arange(S, dtype=np.float64)[:, None] * inv[None, :]  # [S, 32]
    cos = np.cos(fr).T  # [32, S]
    sin = np.sin(fr).T
    cosT = np.concatenate([cos, cos], 0)  # [64, S]
    sgnT = np.concatenate([-sin, sin], 0)  # [64, S]
    cos128 = np.concatenate([cosT, cosT], 0).astype(BF)  # [128, S]
    sgn128 = np.concatenate([sgnT, sgnT], 0).astype(BF)
    p = np.arange(128)[:, None]
    c = np.arange(128)[None, :]
    tri01 = (p <= c).astype(BF)  # [128, 128] 1 on/below diag (key<=query)
    ident = np.eye(128, dtype=np.float32).astype(BF)
    return cos128, sgn128, tri01, ident


def _body(nc, tc, ctx, hid, wq, wk, wv, wo, cosd, sgnd, trid, identd, onesd, outd):
    # ---- constants ----
    cpool = ctx.enter_context(tc.tile_pool(name="const", bufs=1))
    identb = cpool.tile([128, 128], BF16, name="identb", tag="identb")
    nc.gpsimd.dma_start(out=identb[:], in_=identd[:])
    tri01 = cpool.tile([128, 128], BF16, name="tri01", tag="tri01")
    cosb = cpool.tile([128, S], BF16, name="cosb", tag="cosb")
    sgnb = cpool.tile([128, S], BF16, name="sgnb", tag="sgnb")

    # ---- persistent SBUF ----
    # Left stack (bottom->top): const, qTp, wstream, hTp+wkp; the top pools
    # are released (LIFO) late in the window to make room for P6 staging.
    qTpool = ctx.enter_context(tc.tile_pool(name="qTp", bufs=1))
    qT = [qTpool.tile([128, S], BF16, name=f"qT{c}", tag=f"qT{c}") for c in range(16)]
    wsp = ctx.enter_context(tc.tile_pool(name="wstream", bufs=4))
    qrp = ctx.enter_context(tc.tile_pool(name="qrope", bufs=2, side="right"))
    expool = ctx.enter_context(tc.tile_pool(name="expp", bufs=2, side="right"))
    attbp = ctx.enter_context(tc.tile_pool(name="attb", bufs=2, side="right"))
    recp = ctx.enter_context(tc.tile_pool(name="recp", bufs=2, side="right"))
    kpool = ctx.enter_context(tc.tile_pool(name="kTp", bufs=1, side="right"))
    kT = kpool.tile([128, 8 * S], BF16, name="kT", tag="kT")
    vapool = ctx.enter_context(tc.tile_pool(name="vap", bufs=1, side="right"))
    va = [
        vapool.tile([128, 8 * 65], BF16, name=f"va{st}", tag=f"va{st}")
        for st in range(8)
    ]
    atTpool = ctx.enter_context(tc.tile_pool(name="attTp", bufs=1, side="right"))
    attT = [
        atTpool.tile([128, S], BF16, name=f"attT{p}", tag=f"attT{p}") for p in range(16)
    ]
    hT_ctx = contextlib.ExitStack()  # closed late-window after last Q/K matmul
    hTpool = hT_ctx.enter_context(tc.tile_pool(name="hTp", bufs=1))
    hT_big = hTpool.tile([128, 16 * S], BF16, name="hTbig", tag="hTbig")
    hT = [hT_big[:, c * S : (c + 1) * S] for c in range(16)]
    wkpool = hT_ctx.enter_context(tc.tile_pool(name="wkp", bufs=2))
    # hidden staging (released at end of P1) — top of the right stack
    hidp_ctx = contextlib.ExitStack()
    hidp = hidp_ctx.enter_context(tc.tile_pool(name="hidnat", bufs=2, side="right"))

    def _load_w(wsrc, col_lo, pool, tag, eng=None):
        sl = []
        for c in range(2):
            wm = pool.tile([128, 8 * 512], BF16, name="wm", tag=tag)
            (eng or nc.sync).dma_start(
                wm.rearrange("p (t f) -> p t f", t=8),
                wsrc.rearrange("(t p) f -> p t f", p=128)[
                    :, c * 8 : c * 8 + 8, col_lo : col_lo + 512
                ],
            )
            sl += [wm[:, h2 * 512 : (h2 + 1) * 512] for h2 in range(8)]
        return sl

    wq_t = {}
    wo_t = {}

    def _rope(rp, ps, ih, out_sl, use_act):
        """psum [128,512] f32 -> RoPE applied -> out_sl (bf16)."""
        sl = slice(ih * 512, (ih + 1) * 512)
        raw = rp.tile([128, 512], BF16, name="rope_raw", tag="rraw")
        if use_act:
            nc.scalar.copy(raw[:], ps[:])
        else:
            nc.vector.tensor_copy(raw[:], ps[:])
        sh = rp.tile([128, 512], BF16, name="rope_sh", tag="rsh")
        for a in range(4):  # partition quarter a reads quarter a^1
            sc = (a ^ 1) * 32
            eng = nc.sync if a % 2 == 0 else nc.gpsimd
            eng.dma_start(out=sh[a * 32 : (a + 1) * 32, :], in_=raw[sc : sc + 32, :])
        tmp = rp.tile([128, 512], BF16, name="rope_tmp", tag="rtmp")
        nc.vector.tensor_mul(tmp[:], raw[:], cosb[:, sl])
        nc.gpsimd.tensor_mul(sh[:], sh[:], sgnb[:, sl])  # in-place rotate*sign
        nc.vector.tensor_add(out_sl, tmp[:], sh[:])

    def k_dup(kfin, ft, ih):
        b0, b1 = 2 * ft, 2 * ft + 1
        o0 = b0 * S + ih * 512
        o1 = b1 * S + ih * 512
        nc.sync.dma_start(kT[0:64, o0 : o0 + 512], kfin[0:64, :])
        nc.gpsimd.dma_start(out=kT[64:128, o0 : o0 + 512], in_=kfin[0:64, :])
        nc.sync.dma_start(kT[64:128, o1 : o1 + 512], kfin[64:128, :])
        nc.gpsimd.dma_start(out=kT[0:64, o1 : o1 + 512], in_=kfin[64:128, :])

    # ==== Phase 1+2 interleaved: PE-transpose hidden tile t, then V st=t ====
    wv_t = _load_w(wv, 0, wsp, "wst", eng=nc.gpsimd)
    nc.gpsimd.dma_start(out=tri01[:], in_=trid[:])
    nc.gpsimd.dma_start(out=cosb[:], in_=cosd[:])
    nc.gpsimd.dma_start(out=sgnb[:], in_=sgnd[:])
    wkg = {}

    def load_wkg(g):
        wm = wkpool.tile([128, 16 * 128], BF16, name="wkg", tag="wkg")
        nc.sync.dma_start(
            wm.rearrange("p (t f) -> p t f", t=16),
            wk.rearrange("(t p) f -> p t f", p=128)[:, :, g * 128 : (g + 1) * 128],
        )
        wkg[g] = wm

    hT3 = hT_big.rearrange("p (c s) -> p c s", s=S)
    hid_tiles = {}

    def load_hid(t):
        ht = hidp.tile([128, H], BF16, name="hidnat", tag="hidnat")
        eng = nc.sync if t % 2 == 0 else nc.scalar
        if t == 0:
            # quarter loads so the first transposes start ~1us earlier
            for q in range(4):
                eng.dma_start(
                    ht[:, q * 512 : (q + 1) * 512],
                    hid[t * 128 : (t + 1) * 128, q * 512 : (q + 1) * 512],
                )
        else:
            eng.dma_start(ht[:], hid[t * 128 : (t + 1) * 128, :])
        hid_tiles[t] = ht

    load_hid(0)
    load_hid(1)
    with tc.tile_pool(name="tpsum", bufs=2, space="PSUM") as tp, tc.tile_pool(
        name="vpsum", bufs=4, space="PSUM"
    ) as vps:
        for t in range(8):
            ps = tp.tile([128, 2048], BF16, name="tp", tag="tp")
            ht = hid_tiles.pop(t)
            for c in range(16):
                nc.tensor.transpose(
                    ps[:, c * 128 : (c + 1) * 128],
                    ht[:, c * 128 : (c + 1) * 128],
                    identb[:],
                )
            nc.vector.tensor_copy(
                hT3[:, :, t * 128 : (t + 1) * 128],
                ps[:].rearrange("p (c s) -> p c s", s=128),
            )
            if t + 2 < 8:
                load_hid(t + 2)
            st = t
            vp = vps.tile([128, 512], F32, name="vp", tag="vp")
            for hh in range(16):
                nc.tensor.matmul(
                    vp[:],
                    hT[hh][:, st * 128 : (st + 1) * 128],
                    wv_t[hh],
                    start=(hh == 0),
                    stop=(hh == 15),
                )
            va3 = va[st].rearrange("p (g c) -> p g c", c=65)
            nc.scalar.copy(va3[:, :, 0:64], vp[:].rearrange("p (g c) -> p g c", c=64))
            nc.gpsimd.dma_start(
                out=va3[:, :, 64:65],
                in_=onesd[st * 128 : (st + 1) * 128, :].rearrange(
                    "p (g c) -> p g c", c=1
                ),
            )
    hidp_ctx.close()
    wq_t[0] = _load_w(wq, 0, wsp, "wst", eng=nc.scalar)
    load_wkg(0)
    load_wkg(1)
    wq_t[1] = _load_w(wq, 512, wsp, "wst")

    # ---- window psum pools (explicit, for staged release) ----
    kpp = tc.alloc_tile_pool(name="kpp", bufs=1, space="PSUM")
    pvp = tc.alloc_tile_pool(name="pvp", bufs=1, space="PSUM")
    qpp = tc.alloc_tile_pool(name="qpp", bufs=1, space="PSUM")
    scp = tc.alloc_tile_pool(name="scp", bufs=2, space="PSUM")

    def k_mms(g, pi, ktile, parts=range(4)):
        ih = pi // 4
        for k in parts:
            hh = 4 * (pi % 4) + k
            nc.tensor.matmul(
                ktile[:],
                wkg[g][:, hh * 128 : (hh + 1) * 128],
                hT[hh][:, ih * 512 : (ih + 1) * 512],
                start=(hh == 0),
                stop=(hh == 15),
                skip_group_check=True,
            )

    def q_mms(g, c, qtile, part):
        ftl, ih = c // 2, c % 2
        for hh in (2 * part, 2 * part + 1):
            nc.tensor.matmul(
                qtile[:],
                wq_t[g][hh][:, ftl * 128 : (ftl + 1) * 128],
                hT[hh][:, ih * 512 : (ih + 1) * 512],
                start=(hh == 0),
                stop=(hh == 15),
                skip_group_check=True,
            )

    def q_rope(g, c, qtile, use_act=False):
        ftl, ih = c // 2, c % 2
        ft = g * 4 + ftl
        _rope(qrp, qtile, ih, qT[ft][:, ih * 512 : (ih + 1) * 512], use_act)

    # ===== pre-window: K group 0 + Q group 0 chunks 0-2 =====
    kt0 = kpp.tile([128, 512], F32, name="kp", tag="kp")
    for pi in range(4):
        k_mms(0, pi, kt0)
    kfin = qrp.tile([128, 512], BF16, name="kfin", tag="kfin", bufs=1)
    _rope(qrp, kt0, 0, kfin[:], use_act=True)
    k_dup(kfin, 0, 0)

    qt0 = qpp.tile([128, 512], F32, name="qp", tag="qp")
    for part in range(8):
        q_mms(0, 0, qt0, part)
    q_rope(0, 0, qt0, use_act=True)

    kt1 = kpp.tile([128, 512], F32, name="kp", tag="kp")
    for pi in range(4, 8):
        k_mms(0, pi, kt1)
    kfin = qrp.tile([128, 512], BF16, name="kfin", tag="kfin", bufs=1)
    _rope(qrp, kt1, 1, kfin[:], use_act=True)
    k_dup(kfin, 0, 1)

    for c in (1, 2):
        qt0 = qpp.tile([128, 512], F32, name="qp", tag="qp")
        for part in range(8):
            q_mms(0, c, qt0, part)
        q_rope(0, c, qt0, use_act=True)

    # ===== the window: 34 slots =====
    # Q chunk schedule: (0, c) at slot c-3 for c in 3..7; (g, c) at slot
    # 8g-3+c for g in 1,2 and 22+c for g=3 (group 3 shifted one slot later
    # to fill the tail, where the exp stream would otherwise bind).
    qsched = {}
    for c in range(3, 8):
        qsched[c - 3] = (0, c)
    for g in range(1, 3):
        for c in range(8):
            qsched[8 * g - 3 + c] = (g, c)
    for c in range(8):
        qsched[22 + c] = (3, c)

    PV_SCHED = {j: (j,) for j in range(8)}
    ex_tiles = {}
    attb_tiles = {}
    pv_tiles = {}
    k_state = {}
    opsum = None

    for sslot in range(34):
        h = sslot
        hp_ = sslot - 1
        do_head = h < 32
        qch = qsched.get(sslot)
        # K filler: group g_k = sslot//8 + 1 built during group sslot//8
        g_k = sslot // 8 + 1 if do_head and sslot // 8 + 1 <= 3 else None
        pi = sslot % 8
        if do_head:
            g, hl = h // 8, h % 8
            slot64 = (h % 2) * 64
            ft = h // 2
            gkv = h // 4
            if hl == 4 and g + 2 <= 3:
                wq_t[g + 2] = _load_w(wq, (g + 2) * 512, wsp, "wst")
                load_wkg(g + 2)
            if h % 2 == 0:
                attb_tiles[h // 2] = attbp.tile(
                    [128, 1024], BF16, name="attb", tag="attb"
                )
            ex_tiles[h] = []
            qt_ps = None
        for jt in range(8):
            if do_head:
                lo = jt * 128
                sc = scp.tile([128, 1024], F32, name="sc", tag="sc")
                kap = kT[slot64 : slot64 + 64, gkv * S + lo : gkv * S + lo + 128]
                qap = qT[ft][slot64 : slot64 + 64, :]
                if jt < 4:
                    nc.tensor.matmul(
                        sc[:, lo:512], kap, qap[:, lo:512],
                        start=True, stop=True, skip_group_check=True,
                    )
                    nc.tensor.matmul(
                        sc[:, 512:1024], kap, qap[:, 512:1024],
                        start=True, stop=True, skip_group_check=True,
                    )
                else:
                    nc.tensor.matmul(
                        sc[:, lo:1024], kap, qap[:, lo:1024],
                        start=True, stop=True, skip_group_check=True,
                    )
                ex = expool.tile([128, 1024 - lo], BF16, name=f"ex{jt}", tag=f"ex{jt}")
                ex_tiles[h].append(ex)
                nc.scalar.activation(ex[:], sc[:, lo:1024], AF.Exp, scale=0.125)
                nc.gpsimd.tensor_mul(ex[:, 0:128], ex[:, 0:128], tri01[:])
            if qch is not None:
                if jt == 0:
                    qt_ps = qpp.tile([128, 512], F32, name="qp", tag="qp")
                q_mms(qch[0], qch[1], qt_ps, jt)
            # tail filler: first two O-proj psum tiles accumulate in slots
            # 30-31 (reusing the q-chunk and K-projection psum banks)
            if sslot in (30, 31):
                fp = (sslot - 30) * 8 + jt
                if fp == 0:
                    pv_tiles["oe0"] = qpp.tile([128, 512], F32, name="qp", tag="qp")
                    pv_tiles["oe1"] = kpp.tile([128, 512], F32, name="kp", tag="kp")
                if fp < 14:
                    for st_, key in ((0, "oe0"), (1, "oe1")):
                        nc.tensor.matmul(
                            pv_tiles[key][:],
                            attT[fp][:, st_ * 128 : (st_ + 1) * 128],
                            wo_t[0][fp],
                            start=(fp == 0),
                            stop=False,
                            skip_group_check=True,
                        )
            if g_k is not None and jt % 2 == 1:
                if pi % 4 == 0 and jt == 1:
                    k_state[g_k] = kpp.tile([128, 512], F32, name="kp", tag="kp")
                k_mms(g_k, pi, k_state[g_k], parts=(jt // 2,))
            if 0 <= hp_ < 32:
                if jt == 0:
                    pv_tiles[hp_] = pvp.tile([128, 1024], F32, name="pv", tag="pv")
                pv = pv_tiles[hp_]
                gkv_p = hp_ // 4
                # front-loaded: all 36 PV matmuls land in jts 0..4 so both
                # normalize halves (jt 2 / jt 5) finish before slot end and
                # the pv psum bank is free for the next head's first matmul
                for it in PV_SCHED.get(jt, ()):
                    for jt2 in range(it + 1):
                        exs = ex_tiles[hp_][jt2]
                        nc.tensor.matmul(
                            pv[:, it * 128 : it * 128 + 65],
                            exs[:, (it - jt2) * 128 : (it - jt2) * 128 + 128],
                            va[jt2].rearrange("p (g c) -> p g c", c=65)[:, gkv_p, :],
                            start=(jt2 == 0),
                            stop=(jt2 == it),
                            skip_group_check=True,
                        )
            # attT DMA-transposes for pair p spread over slot 2p+3
            if sslot >= 3 and sslot % 2 == 1:
                p = (sslot - 3) // 2
                if p <= 15 and p in attb_tiles:
                    ab = attb_tiles[p]
                    nc.sync.dma_start_transpose(
                        attT[p][:, jt * 128 : (jt + 1) * 128],
                        ab[:, jt * 128 : (jt + 1) * 128],
                    )
        # ---- slot end: normalize, K rope, Q rope ----
        if 0 <= hp_ < 32:
            pv = pv_tiles.pop(hp_)
            pv3 = pv.rearrange("p (i c) -> p i c", c=128)
            rec = recp.tile([128, 8], F32, name="rec", tag="rec")
            nc.vector.reciprocal_approx_fast(
                rec[:], pv3[:, :, 64:65].rearrange("p i c -> p (i c)")
            )
            hs = hp_ % 2
            ab = attb_tiles[hp_ // 2]
            for it in range(8):
                nc.vector.tensor_scalar_mul(
                    ab[:, it * 128 + hs * 64 : it * 128 + hs * 64 + 64],
                    pv[:, it * 128 : it * 128 + 64],
                    rec[:, it : it + 1],
                )
            del ex_tiles[hp_]
        if g_k is not None and pi % 4 == 3:
            ih = pi // 4
            kfin = qrp.tile([128, 512], BF16, name="kfin", tag="kfin", bufs=1)
            _rope(qrp, k_state[g_k], ih, kfin[:], use_act=False)
            k_dup(kfin, g_k, ih)
        if qch is not None:
            q_rope(qch[0], qch[1], qt_ps, use_act=False)
        if sslot >= 3 and sslot % 2 == 1:
            p = (sslot - 3) // 2
            if p <= 15 and p in attb_tiles:
                attb_tiles.pop(p)
        if sslot == 27:
            wo_t[0] = _load_w(wo, 0, wsp, "wst")
        if sslot == 29:
            hT_ctx.close()  # free hT + wk; make room for P6 staging
            wo_t[1] = _load_w(wo, 512, wsp, "wst")
        if sslot == 31:
            scp.release()  # free 4 psum banks for the O-projection
            opsum = tc.alloc_tile_pool(name="opsum", bufs=4, space="PSUM")

    # ================= Phase 6: O projection ================
    with tc.tile_pool(name="osb", bufs=4) as osbp:
        for ho in range(4):
            if ho + 2 <= 3:
                wo_t[ho + 2] = _load_w(wo, (ho + 2) * 512, wsp, "wst")
            for st in range(8):
                if ho == 0 and st in (0, 1):
                    # finish the tiles started as tail filler (fp 0..13)
                    ps = pv_tiles.pop(f"oe{st}")
                    fps = range(14, 16)
                    first = False
                else:
                    ps = opsum.tile([128, 512], F32, name="op", tag="op")
                    fps = range(16)
                    first = True
                for fp in fps:
                    nc.tensor.matmul(
                        ps[:],
                        attT[fp][:, st * 128 : (st + 1) * 128],
                        wo_t[ho][fp],
                        start=(first and fp == 0),
                        stop=(fp == 15),
                        skip_group_check=True,
                    )
                ob = osbp.tile([128, 512], F32, name="ob", tag="ob")
                last = ho == 3 and st == 7
                if last:
                    # split the final drain so the copy+DMA pipeline overlaps
                    nc.scalar.copy(ob[:, 0:256], ps[:, 0:256])
                    nc.gpsimd.dma_start(
                        out=outd[st * 128 : (st + 1) * 128, ho * 512 : ho * 512 + 256],
                        in_=ob[:, 0:256],
                    )
                    nc.scalar.copy(ob[:, 256:512], ps[:, 256:512])
                    nc.sync.dma_start(
                        out=outd[st * 128 : (st + 1) * 128, ho * 512 + 256 : ho * 512 + 512],
                        in_=ob[:, 256:512],
                    )
                else:
                    nc.scalar.copy(ob[:], ps[:])
                    eng = nc.gpsimd if st % 2 == 0 else nc.sync
                    eng.dma_start(
                        out=outd[st * 128 : (st + 1) * 128, ho * 512 : (ho + 1) * 512],
                        in_=ob[:],
                    )
    opsum.release()
    qpp.release()
    pvp.release()
    kpp.release()


def _build(niter=1):
    nc = bacc.Bacc(None, target_bir_lowering=False)
    hid = nc.declare_dram_parameter("hidden_states", [S, H], BF16, isOutput=False)
    wq = nc.declare_dram_parameter("Wq", [H, NQ * D], BF16, isOutput=False)
    wk = nc.declare_dram_parameter("Wk", [H, NKV * D], BF16, isOutput=False)
    wv = nc.declare_dram_parameter("Wv", [H, NKV * D], BF16, isOutput=False)
    wo = nc.declare_dram_parameter("Wo", [NQ * D, H], BF16, isOutput=False)
    cosd = nc.declare_dram_parameter("rope_cos", [128, S], BF16, isOutput=False)
    sgnd = nc.declare_dram_parameter("rope_sgnsin", [128, S], BF16, isOutput=False)
    trid = nc.declare_dram_parameter("tri01", [128, 128], BF16, isOutput=False)
    identd = nc.declare_dram_parameter("ident", [128, 128], BF16, isOutput=False)
    onesd = nc.declare_dram_parameter("ones_col", [S, 8], BF16, isOutput=False)
    outd = nc.declare_dram_parameter("out", [S, H], F32, isOutput=True)

    with tile.TileContext(nc) as tc:
        for _ in range(niter):
            with contextlib.ExitStack() as ctx:
                _body(nc, tc, ctx, hid, wq, wk, wv, wo, cosd, sgnd, trid, identd, onesd, outd)
    nc.compile()
    return nc


_CACHE = {}


def _get_nc(niter=1):
    if niter not in _CACHE:
        _CACHE[niter] = _build(niter)
    return _CACHE[niter]


def _in_maps(inputs):
    cos128, sgn128, tri01, ident = _tables()
    hidden = np.asarray(inputs["hidden_states"], dtype=np.float32).astype(BF)
    base = {
        "Wq": np.asarray(inputs["Wq"], dtype=np.float32).astype(BF),
        "Wk": np.asarray(inputs["Wk"], dtype=np.float32).astype(BF),
        "Wv": np.asarray(inputs["Wv"], dtype=np.float32).astype(BF),
        "Wo": np.asarray(inputs["Wo"], dtype=np.float32).astype(BF),
        "rope_cos": cos128,
        "rope_sgnsin": sgn128,
        "tri01": tri01,
        "ident": ident,
        "ones_col": np.ones((S, 8), BF),
    }
    return [dict(base, hidden_states=hidden[i]) for i in range(B)]


def kernel(**inputs):
    nc = _get_nc(1)
    res = run_bass_kernel_spmd(nc, _in_maps(inputs), core_ids=list(range(8)))
    return np.stack([res.results[i]["out"] for i in range(B)]).astype(np.float32)
